# revision 7
# baseline (speedup 1.0000x reference)
"""ALiBi attention on 8 TRN2 NeuronCores.

Sharding: 8 cores = 4 batches x 2 query-halves (pure data parallel, zero
collectives). Each core computes output rows [b, q0:q0+1024] end-to-end:
K/V projections are duplicated across the 2 cores of a batch (~25% extra
FLOPs) in exchange for no cross-core communication.

Per-core layout (all matmuls bf16 into f32 PSUM):
  - host pre-transposes x and weights; wq rows of head h pre-scaled by
    1/(8*slope_h) so the ALiBi slope becomes the ACT exp's scale constant
  - S'^T tile = matmul(lhsT=KT_h, rhs=QT'_h)   (k on partitions, q free)
  - DVE adds the -|k-q| distance tile (f32, SBUF-resident: only 20
    distinct (k,q)-tiles per core since content depends on k-q only;
    graph indexes them uniformly, host bakes the per-core q-offset)
  - ACT exp(scale=slope_h) -> bf16 P^T tile
  - matmul(lhsT=[V_h | 1], rhs=P^T) accumulates O^T plus a denominator
    row in PSUM; DVE reciprocal + DMA partition-broadcast + DVE multiply
    normalizes into OT
  - out-projection contracts OT with woT, ACT adds bo, DMA out f32
"""

import math
import sys

import numpy as np

sys.path.insert(0, "/opt/trn_rl_repo")

D_MODEL = 1024
NUM_HEADS = 16
HEAD_DIM = 64
B, T = 4, 2048
TQ = 1024  # queries per core
N_CORES = 8
NDT = D_MODEL // 128  # 8 tiles of 128
NKT = T // 128  # 16 k-tiles
NU = 20  # distinct dist tiles: u = kt - 4*qc + 4 in [0, 20)


def _get_slopes(n_heads: int) -> np.ndarray:
    def pow2_slopes(n):
        start = 2 ** (-(2 ** (-(math.log2(n) - 3))))
        return [start * start**i for i in range(n)]

    if math.log2(n_heads).is_integer():
        return np.array(pow2_slopes(n_heads), dtype=np.float32)
    closest = 2 ** math.floor(math.log2(n_heads))
    slopes = pow2_slopes(closest)
    slopes += pow2_slopes(2 * closest)[0::2][: n_heads - closest]
    return np.array(slopes[:n_heads], dtype=np.float32)


_BUILT = None


def _build():
    global _BUILT
    if _BUILT is not None:
        return _BUILT

    from contextlib import ExitStack

    import concourse.bass as bass
    import concourse.tile as tile
    from concourse import bacc, mybir

    f32 = mybir.dt.float32
    bf16 = mybir.dt.bfloat16
    Exp = mybir.ActivationFunctionType.Exp
    Ident = mybir.ActivationFunctionType.Identity

    slopes = _get_slopes(NUM_HEADS)

    nc = bacc.Bacc(None, target_bir_lowering=False)

    xT_d = nc.dram_tensor("xT", [NDT, 128, T], bf16, kind="ExternalInput")
    xqT_d = nc.dram_tensor("xqT", [NDT, 128, TQ], bf16, kind="ExternalInput")
    wqT_d = nc.dram_tensor("wqT", [NDT, 128, D_MODEL], bf16, kind="ExternalInput")
    wkT_d = nc.dram_tensor("wkT", [NDT, 128, D_MODEL], bf16, kind="ExternalInput")
    wvT_d = nc.dram_tensor("wvT", [NDT, 128, D_MODEL], bf16, kind="ExternalInput")
    woT_d = nc.dram_tensor("woT", [NDT, 128, D_MODEL], bf16, kind="ExternalInput")
    dist_d = nc.dram_tensor("dist", [NU, 128, 512], f32, kind="ExternalInput")
    bo_d = nc.dram_tensor("bo_t", [128, NDT], f32, kind="ExternalInput")
    ones_d = nc.dram_tensor("onesc", [128, 256], bf16, kind="ExternalInput")
    out_d = nc.dram_tensor("out", [NDT, 128, TQ], f32, kind="ExternalOutput")

    with tile.TileContext(nc) as tc, ExitStack() as ctx:
        # ---- persistent SBUF tensors (live across phases) ----
        persist = ctx.enter_context(tc.tile_pool(name="persist", bufs=1))
        QT = persist.tile([128, NDT * TQ], bf16, tag="QT")  # 16KB/part
        KT = persist.tile([128, NDT * T], bf16, tag="KT")  # 32KB/part
        VP = persist.tile([128, NKT * (NUM_HEADS * 65)], bf16, tag="VP")  # 32.5KB
        OT = persist.tile([128, NDT * TQ], bf16, tag="OT")  # 16KB
        bo_sb = persist.tile([128, NDT], f32, tag="bo")
        ones_sb = persist.tile([128, 256], bf16, tag="ones")
        nc.sync.dma_start(out=bo_sb[:], in_=bo_d[:])
        nc.sync.dma_start(out=ones_sb[:], in_=ones_d[:])
        # V_aug ones columns: VP[:, kt*1040 + h*65 + 64] = 1
        vp4 = VP[:].rearrange(
            "p (t h c) -> p t h c", t=NKT, h=NUM_HEADS, c=65
        )
        nc.sync.dma_start(
            out=vp4[:, :, :, 64],
            in_=ones_sb[:].rearrange("p (a b) -> p a b", a=16),
        )

        # ---- phase 1a: K and V projections (consume xT, wkT, wvT) ----
        with nc.named_scope("proj_kv"):
            with tc.tile_pool(name="ph1a", bufs=1) as ph1a, tc.tile_pool(
                name="ps1", bufs=3, space="PSUM"
            ) as ps1:
                xT = ph1a.tile([128, NDT * T], bf16, tag="xT")
                wkT = ph1a.tile([128, NDT * D_MODEL], bf16, tag="wkT")
                wvT = ph1a.tile([128, NDT * D_MODEL], bf16, tag="wvT")
                for et in range(NDT):
                    nc.sync.dma_start(out=xT[:, et * T : (et + 1) * T], in_=xT_d[et])
                    nc.sync.dma_start(
                        out=wkT[:, et * D_MODEL : (et + 1) * D_MODEL], in_=wkT_d[et]
                    )
                    nc.sync.dma_start(
                        out=wvT[:, et * D_MODEL : (et + 1) * D_MODEL], in_=wvT_d[et]
                    )
                # KT[dt*128+m, t] = sum_e wk[dt*128+m, e] x[t, e]
                for dt in range(NDT):
                    for tc_ in range(T // 512):
                        ps = ps1.tile([128, 512], f32, tag="ps1")
                        for et in range(NDT):
                            nc.tensor.matmul(
                                ps[:],
                                lhsT=wkT[:, et * D_MODEL + dt * 128 : et * D_MODEL + dt * 128 + 128],
                                rhs=xT[:, et * T + tc_ * 512 : et * T + tc_ * 512 + 512],
                                start=(et == 0),
                                stop=(et == NDT - 1),
                            )
                        nc.any.tensor_copy(
                            out=KT[:, dt * T + tc_ * 512 : dt * T + tc_ * 512 + 512],
                            in_=ps[:],
                        )
                # V natural: V[t, dout] = sum_e x[t, e] wv[dout, e]
                # psum (128=t, 512=8 heads), write strided into VP (65-wide blocks)
                for tt in range(NKT):
                    for dc in range(2):
                        ps = ps1.tile([128, 512], f32, tag="ps1")
                        for et in range(NDT):
                            nc.tensor.matmul(
                                ps[:],
                                lhsT=xT[:, et * T + tt * 128 : et * T + tt * 128 + 128],
                                rhs=wvT[:, et * D_MODEL + dc * 512 : et * D_MODEL + dc * 512 + 512],
                                start=(et == 0),
                                stop=(et == NDT - 1),
                            )
                        nc.any.tensor_copy(
                            out=vp4[:, tt, dc * 8 : (dc + 1) * 8, 0:64],
                            in_=ps[:].rearrange("p (h c) -> p h c", h=8),
                        )

        # ---- phase 1b: Q projection on the core's query half ----
        with nc.named_scope("proj_q"):
            with tc.tile_pool(name="ph1b", bufs=1) as ph1b, tc.tile_pool(
                name="ps2", bufs=3, space="PSUM"
            ) as ps2:
                xqT = ph1b.tile([128, NDT * TQ], bf16, tag="xqT")
                wqT = ph1b.tile([128, NDT * D_MODEL], bf16, tag="wqT")
                for et in range(NDT):
                    nc.sync.dma_start(out=xqT[:, et * TQ : (et + 1) * TQ], in_=xqT_d[et])
                    nc.sync.dma_start(
                        out=wqT[:, et * D_MODEL : (et + 1) * D_MODEL], in_=wqT_d[et]
                    )
                for dt in range(NDT):
                    for qc in range(TQ // 512):
                        ps = ps2.tile([128, 512], f32, tag="ps2")
                        for et in range(NDT):
                            nc.tensor.matmul(
                                ps[:],
                                lhsT=wqT[:, et * D_MODEL + dt * 128 : et * D_MODEL + dt * 128 + 128],
                                rhs=xqT[:, et * TQ + qc * 512 : et * TQ + qc * 512 + 512],
                                start=(et == 0),
                                stop=(et == NDT - 1),
                            )
                        nc.any.tensor_copy(
                            out=QT[:, dt * TQ + qc * 512 : dt * TQ + qc * 512 + 512],
                            in_=ps[:],
                        )

        # ---- phase 2: attention ----
        with nc.named_scope("attn"):
            with tc.tile_pool(name="distp", bufs=1) as distp, tc.tile_pool(
                name="scr", bufs=4
            ) as scr, tc.tile_pool(name="ptp", bufs=4) as ptp, tc.tile_pool(
                name="small", bufs=4
            ) as small, tc.tile_pool(
                name="ps_s", bufs=3, space="PSUM"
            ) as ps_s, tc.tile_pool(
                name="ps_o", bufs=2, space="PSUM"
            ) as ps_o:
                dist_t = []
                for u in range(NU):
                    dt_tile = distp.tile([128, 512], f32, tag=f"dist{u}")
                    nc.sync.dma_start(out=dt_tile[:], in_=dist_d[u])
                    dist_t.append(dt_tile)
                for h in range(NUM_HEADS):
                    sl = float(slopes[h])
                    par = (h % 2) * 64
                    dt = h // 2
                    for qc in range(TQ // 512):
                        o_ps = ps_o.tile([65, 512], f32, tag="o")
                        for kt in range(NKT):
                            u = kt - 4 * qc + 4
                            s_ps = ps_s.tile([128, 512], f32, tag="s")
                            nc.tensor.matmul(
                                s_ps[:],
                                lhsT=KT[par : par + 64, dt * T + kt * 128 : dt * T + kt * 128 + 128],
                                rhs=QT[par : par + 64, dt * TQ + qc * 512 : dt * TQ + qc * 512 + 512],
                                start=True,
                                stop=True,
                            )
                            sscr = scr.tile([128, 512], f32, tag="sscr")
                            nc.vector.tensor_add(sscr[:], s_ps[:], dist_t[u][:])
                            pt = ptp.tile([128, 512], bf16, tag="pt")
                            nc.scalar.activation(pt[:], sscr[:], Exp, bias=0.0, scale=sl)
                            nc.tensor.matmul(
                                o_ps[:],
                                lhsT=VP[:, kt * 1040 + h * 65 : kt * 1040 + h * 65 + 65],
                                rhs=pt[:],
                                start=(kt == 0),
                                stop=(kt == NKT - 1),
                            )
                        rec = small.tile([1, 512], f32, tag="rec")
                        nc.vector.reciprocal(rec[:], o_ps[64:65, :])
                        rec_bf = small.tile([1, 512], bf16, tag="recbf")
                        nc.scalar.copy(rec_bf[:], rec[:])
                        rbc_ps = ps_o.tile([64, 512], f32, tag="rbc")
                        nc.tensor.matmul(
                            rbc_ps[:],
                            lhsT=ones_sb[0:1, 0:64],
                            rhs=rec_bf[:],
                            start=True,
                            stop=True,
                        )
                        rbc_sb = small.tile([64, 512], f32, tag="rbcsb")
                        nc.scalar.copy(rbc_sb[:], rbc_ps[:])
                        nc.vector.tensor_mul(
                            OT[par : par + 64, dt * TQ + qc * 512 : dt * TQ + qc * 512 + 512],
                            o_ps[0:64, :],
                            rbc_sb[:],
                        )

        # ---- phase 3: output projection + bias ----
        with nc.named_scope("outproj"):
            with tc.tile_pool(name="ph3", bufs=1) as ph3, tc.tile_pool(
                name="ph3s", bufs=3
            ) as ph3s, tc.tile_pool(name="ps3", bufs=3, space="PSUM") as ps3:
                woT = ph3.tile([128, NDT * D_MODEL], bf16, tag="woT")
                for et in range(NDT):
                    nc.sync.dma_start(
                        out=woT[:, et * D_MODEL : (et + 1) * D_MODEL], in_=woT_d[et]
                    )
                for jt in range(NDT):
                    for qc in range(TQ // 512):
                        ps = ps3.tile([128, 512], f32, tag="ps3")
                        for dm in range(NDT):
                            nc.tensor.matmul(
                                ps[:],
                                lhsT=woT[:, dm * D_MODEL + jt * 128 : dm * D_MODEL + jt * 128 + 128],
                                rhs=OT[:, dm * TQ + qc * 512 : dm * TQ + qc * 512 + 512],
                                start=(dm == 0),
                                stop=(dm == NDT - 1),
                            )
                        osb = ph3s.tile([128, 512], f32, tag="osb")
                        nc.scalar.activation(
                            osb[:], ps[:], Ident, bias=bo_sb[:, jt : jt + 1], scale=1.0
                        )
                        nc.sync.dma_start(
                            out=out_d[jt][:, qc * 512 : qc * 512 + 512], in_=osb[:]
                        )

    nc.finalize()
    _BUILT = nc
    return nc


def _pack(mat: np.ndarray, np_bf16) -> np.ndarray:
    """(n*128, C) f32 -> (n, 128, C) bf16."""
    n = mat.shape[0] // 128
    return np.ascontiguousarray(mat.reshape(n, 128, mat.shape[1]).astype(np_bf16))


def _install_trace_hook():
    """Recreate the NTFF profiling hook this container's image lacks.

    Mirrors trn_boot.py's _ntff_profile_via_ctypes against
    /opt/axon/libaxon_pjrt.so and registers it under the module path
    concourse.bass_utils expects. Also neutralizes upload_artifacts
    (no bucket access here)."""
    import contextlib
    import ctypes
    import sys
    import types

    if "antenv.axon_hooks" in sys.modules:
        return True
    so_path = "/opt/axon/libaxon_pjrt.so"
    try:
        lib = ctypes.CDLL(so_path)
    except OSError:
        return False
    if not hasattr(lib, "axon_start_nrt_profile"):
        return False
    lib.axon_start_nrt_profile.argtypes = [
        ctypes.POINTER(ctypes.c_int64),
        ctypes.c_size_t,
    ]
    lib.axon_start_nrt_profile.restype = ctypes.c_int64
    lib.axon_stop_nrt_profile.argtypes = [ctypes.c_char_p]
    lib.axon_stop_nrt_profile.restype = ctypes.c_int64

    @contextlib.contextmanager
    def _hook(output_dir, device_ids):
        import jax

        jax.devices()
        if device_ids:
            ids = (ctypes.c_int64 * len(device_ids))(*device_ids)
            rc = lib.axon_start_nrt_profile(ids, len(device_ids))
        else:
            rc = lib.axon_start_nrt_profile(None, 0)
        if rc != 0:
            raise RuntimeError(f"axon_start_nrt_profile rc={rc}")
        try:
            yield
        finally:
            n = lib.axon_stop_nrt_profile(str(output_dir).encode())
            print(f"profile: {n} file(s) written to {output_dir}")

    mod = types.ModuleType("antenv.axon_hooks")
    mod.get_axon_ntff_profile_hook = lambda: _hook
    mod.set_axon_ntff_profile_hook = lambda h: None
    sys.modules["antenv.axon_hooks"] = mod
    import antenv

    antenv.axon_hooks = mod

    import concourse.bass_utils as bu

    bu.upload_artifacts = lambda tmpdir: str(tmpdir)
    return True


def kernel(x, wq, wk, wv, wo, bo):
    import ml_dtypes

    from concourse.bass_utils import run_bass_kernel_spmd

    bf = ml_dtypes.bfloat16
    x = np.asarray(x, np.float32)
    wq = np.asarray(wq, np.float32)
    wk = np.asarray(wk, np.float32)
    wv = np.asarray(wv, np.float32)
    wo = np.asarray(wo, np.float32)
    bo = np.asarray(bo, np.float32)

    slopes = _get_slopes(NUM_HEADS)
    # fold 1/(8*slope_h) into wq rows of head h
    inv = np.repeat(1.0 / (8.0 * slopes), HEAD_DIM).astype(np.float32)
    wq_s = wq * inv[:, None]

    wqT_p = _pack(wq_s.T, bf)
    wkT_p = _pack(wk.T, bf)
    wvT_p = _pack(wv.T, bf)
    woT_p = _pack(wo.T, bf)
    bo_t = np.ascontiguousarray(bo.reshape(NDT, 128).T.astype(np.float32))
    onesc = np.ones((128, 256), dtype=bf)

    nc = _build()

    in_maps = []
    for core in range(N_CORES):
        b, half = core // 2, core % 2
        q0 = half * TQ
        xT_b = x[b].T  # (1024, 2048)
        # dist tile u: [p, j] = -|128*(u-4) - q0 + p - j|
        u_ = np.arange(NU)[:, None, None]
        p_ = np.arange(128)[None, :, None]
        j_ = np.arange(512)[None, None, :]
        dist = -np.abs(128.0 * (u_ - 4) - q0 + p_ - j_).astype(np.float32)
        in_maps.append(
            {
                "xT": _pack(xT_b, bf),
                "xqT": _pack(np.ascontiguousarray(xT_b[:, q0 : q0 + TQ]), bf),
                "wqT": wqT_p,
                "wkT": wkT_p,
                "wvT": wvT_p,
                "woT": woT_p,
                "dist": np.ascontiguousarray(dist.astype(np.float32)),
                "bo_t": bo_t,
                "onesc": onesc,
            }
        )

    import os

    trace = os.environ.get("KERNEL_TRACE", "0") == "1"
    if trace and not _install_trace_hook():
        trace = False
    tmpdir = os.environ.get("KERNEL_TRACE_DIR") or None
    res = run_bass_kernel_spmd(
        nc, in_maps, core_ids=list(range(N_CORES)), trace=trace, tmpdir=tmpdir
    )
    if trace and res.exec_time_ns is not None:
        print(f"HW exec time: {res.exec_time_ns} ns")
        kernel.last_exec_time_ns = res.exec_time_ns
        kernel.last_results = res

    out = np.empty((B, T, D_MODEL), dtype=np.float32)
    for core in range(N_CORES):
        b, half = core // 2, core % 2
        o = np.asarray(res.results[core]["out"])  # (8, 128, 1024)
        out[b, half * TQ : (half + 1) * TQ, :] = o.reshape(D_MODEL, TQ).T
    return out


# revision 11
# speedup vs baseline: 1.5228x; 1.5228x over previous
"""ALiBi attention on 8 TRN2 NeuronCores.

Sharding: 8 cores = 4 batches x 2 query-halves (pure data parallel, zero
collectives). Each core computes output rows [b, q0:q0+1024] end-to-end;
K/V projections are duplicated across the 2 cores of a batch (~25% extra
FLOPs) in exchange for no cross-core communication.

The K/V time axis is ROLLED by q0 on the host, so in core-local
coordinates the queries are always columns [0, 1024) of xT and the
ALiBi band is always centered the same way — this keeps the banded
k-tile schedule identical across cores (SPMD) while the per-core dist
tile *contents* (baked on host) carry the actual |k-q| geometry,
including the wrap-around.

Per-core pipeline (matmuls bf16 into f32 PSUM):
  - wq rows of head h pre-scaled by 1/(8*slope_h) on host, so the exp
    becomes exp(slope_h * (S' + dist)) with a scalar scale per head
  - per (head, k-tile): S'^T for both 512-wide q-chunks lands in one
    (128,1024) 2-bank PSUM tile; one DVE add of the dist tile; one ACT
    exp -> bf16 P^T; two PV matmuls accumulate O^T + a denominator row
    (ones column in V) into a (65,1024) PSUM tile
  - k-tiles with slope_h*min|dist| > 14 are skipped (exp underflow)
  - PV matmuls are emitted LAG iterations behind S' so TensorE never
    stalls on the DVE/ACT chain
  - normalize via approx reciprocal + ones-matmul partition broadcast,
    out-projection contracts with woT, ACT adds bo, DMA out f32
"""

import math
import sys

import numpy as np

sys.path.insert(0, "/opt/trn_rl_repo")

D_MODEL = 1024
NUM_HEADS = 16
HEAD_DIM = 64
B, T = 4, 2048
TQ = 1024  # queries per core
N_CORES = 8
NDT = D_MODEL // 128  # 8 tiles of 128
NKT = T // 128  # 16 k-tiles
LAG = 2  # software-pipeline depth (PV trails S' by LAG k-tiles)


def _get_slopes(n_heads: int) -> np.ndarray:
    def pow2_slopes(n):
        start = 2 ** (-(2 ** (-(math.log2(n) - 3))))
        return [start * start**i for i in range(n)]

    if math.log2(n_heads).is_integer():
        return np.array(pow2_slopes(n_heads), dtype=np.float32)
    closest = 2 ** math.floor(math.log2(n_heads))
    slopes = pow2_slopes(closest)
    slopes += pow2_slopes(2 * closest)[0::2][: n_heads - closest]
    return np.array(slopes[:n_heads], dtype=np.float32)


_SLOPES = _get_slopes(NUM_HEADS)


def _ktset(h: int) -> list[int]:
    """k-tiles whose ALiBi factor is not uniformly ~0 for head h.

    Local-coordinate circular window [qlo - D, qhi + D) around each
    q-chunk; D chosen so omitted tiles have slope*|dist| > 14
    (exp(-14) ~ 8e-7, far below bf16 resolution of P)."""
    D = 14.0 / float(_SLOPES[h])
    if 512 + 2 * D >= T:
        return list(range(NKT))
    s = set()
    for qc in range(2):
        lo, hi = qc * 512 - D, qc * 512 + 512 + D
        a, b = lo % T, hi % T
        for kt in range(NKT):
            k0, k1 = kt * 128, kt * 128 + 128
            if a < b:
                hit = k0 < b and k1 > a
            else:
                hit = k0 < b or k1 > a
            if hit:
                s.add(kt)
    return sorted(s)


_BUILT = None


def _build():
    global _BUILT
    if _BUILT is not None:
        return _BUILT

    from contextlib import ExitStack

    import concourse.bass as bass
    import concourse.tile as tile
    from concourse import bacc, mybir

    f32 = mybir.dt.float32
    bf16 = mybir.dt.bfloat16
    Exp = mybir.ActivationFunctionType.Exp
    Ident = mybir.ActivationFunctionType.Identity

    nc = bacc.Bacc(None, target_bir_lowering=False)

    xT_d = nc.dram_tensor("xT", [NDT, 128, T], bf16, kind="ExternalInput")
    wqT_d = nc.dram_tensor("wqT", [NDT, 128, D_MODEL], bf16, kind="ExternalInput")
    wkT_d = nc.dram_tensor("wkT", [NDT, 128, D_MODEL], bf16, kind="ExternalInput")
    wvT_d = nc.dram_tensor("wvT", [NDT, 128, D_MODEL], bf16, kind="ExternalInput")
    woT_d = nc.dram_tensor("woT", [NDT, 128, D_MODEL], bf16, kind="ExternalInput")
    dist_d = nc.dram_tensor("dist2", [NKT, 128, 1024], f32, kind="ExternalInput")
    bo_d = nc.dram_tensor("bo_t", [128, NDT], f32, kind="ExternalInput")
    ones_d = nc.dram_tensor("onesc", [128, 256], bf16, kind="ExternalInput")
    out_d = nc.dram_tensor("out", [NDT, 128, TQ], f32, kind="ExternalOutput")

    with tile.TileContext(nc) as tc, ExitStack() as ctx:
        # ---- persistent SBUF tensors ----
        persist = ctx.enter_context(tc.tile_pool(name="persist", bufs=1))
        QT = persist.tile([128, NDT * TQ], bf16, tag="QT")  # 16KB/part
        KT = persist.tile([128, NDT * T], bf16, tag="KT")  # 32KB/part
        VP = persist.tile([128, NKT * (NUM_HEADS * 65)], bf16, tag="VP")  # 32.5KB
        OT = persist.tile([128, NDT * TQ], bf16, tag="OT")  # 16KB
        bo_sb = persist.tile([128, NDT], f32, tag="bo")
        ones_sb = persist.tile([128, 256], bf16, tag="ones")
        nc.sync.dma_start(out=bo_sb[:], in_=bo_d[:])
        nc.sync.dma_start(out=ones_sb[:], in_=ones_d[:])
        vp4 = VP[:].rearrange("p (t h c) -> p t h c", t=NKT, h=NUM_HEADS, c=65)
        nc.sync.dma_start(
            out=vp4[:, :, :, 64],
            in_=ones_sb[:].rearrange("p (a b) -> p a b", a=16),
        )

        # ---- phase 1: QKV projections (one pool; x freed after) ----
        with nc.named_scope("proj"):
            with tc.tile_pool(name="ph1", bufs=1) as ph1, tc.tile_pool(
                name="ps1", bufs=3, space="PSUM"
            ) as ps1:
                xT = ph1.tile([128, NDT * T], bf16, tag="xT")
                wkT = ph1.tile([128, NDT * D_MODEL], bf16, tag="wkT")
                wvT = ph1.tile([128, NDT * D_MODEL], bf16, tag="wvT")
                wqT = ph1.tile([128, NDT * D_MODEL], bf16, tag="wqT")
                # DMA order matters: K-proj (xT + wkT) unblocks first
                for et in range(NDT):
                    nc.sync.dma_start(out=xT[:, et * T : (et + 1) * T], in_=xT_d[et])
                    nc.sync.dma_start(
                        out=wkT[:, et * D_MODEL : (et + 1) * D_MODEL], in_=wkT_d[et]
                    )
                for et in range(NDT):
                    nc.sync.dma_start(
                        out=wvT[:, et * D_MODEL : (et + 1) * D_MODEL], in_=wvT_d[et]
                    )
                for et in range(NDT):
                    nc.sync.dma_start(
                        out=wqT[:, et * D_MODEL : (et + 1) * D_MODEL], in_=wqT_d[et]
                    )

                # KT[dt*128+m, t] = sum_e wk[dt*128+m, e] x[t, e]
                for dt in range(NDT):
                    for tc2 in range(T // 1024):
                        ps = ps1.tile([128, 1024], f32, tag="ps1")
                        for half in range(2):
                            for et in range(NDT):
                                nc.tensor.matmul(
                                    ps[:, half * 512 : half * 512 + 512],
                                    lhsT=wkT[:, et * D_MODEL + dt * 128 : et * D_MODEL + dt * 128 + 128],
                                    rhs=xT[:, et * T + tc2 * 1024 + half * 512 : et * T + tc2 * 1024 + half * 512 + 512],
                                    start=(et == 0),
                                    stop=(et == NDT - 1),
                                )
                        nc.scalar.copy(
                            KT[:, dt * T + tc2 * 1024 : dt * T + tc2 * 1024 + 1024],
                            ps[:],
                        )
                # V natural: V[t, dout] = sum_e x[t, e] wv[dout, e]
                for tt in range(NKT):
                    ps = ps1.tile([128, 1024], f32, tag="ps1")
                    for half in range(2):
                        for et in range(NDT):
                            nc.tensor.matmul(
                                ps[:, half * 512 : half * 512 + 512],
                                lhsT=xT[:, et * T + tt * 128 : et * T + tt * 128 + 128],
                                rhs=wvT[:, et * D_MODEL + half * 512 : et * D_MODEL + half * 512 + 512],
                                start=(et == 0),
                                stop=(et == NDT - 1),
                            )
                    nc.scalar.copy(
                        vp4[:, tt, :, 0:64],
                        ps[:].rearrange("p (h c) -> p h c", h=16),
                    )
                # Q on the local query half = rolled columns [0, 1024)
                for dt in range(NDT):
                    ps = ps1.tile([128, 1024], f32, tag="ps1")
                    for half in range(2):
                        for et in range(NDT):
                            nc.tensor.matmul(
                                ps[:, half * 512 : half * 512 + 512],
                                lhsT=wqT[:, et * D_MODEL + dt * 128 : et * D_MODEL + dt * 128 + 128],
                                rhs=xT[:, et * T + half * 512 : et * T + half * 512 + 512],
                                start=(et == 0),
                                stop=(et == NDT - 1),
                            )
                    nc.scalar.copy(QT[:, dt * TQ : dt * TQ + 1024], ps[:])

        # ---- phase 2: banded attention, software-pipelined ----
        with nc.named_scope("attn"):
            with tc.tile_pool(name="distp", bufs=1) as distp, tc.tile_pool(
                name="scr", bufs=3
            ) as scr, tc.tile_pool(name="ptp", bufs=3) as ptp, tc.tile_pool(
                name="small", bufs=1
            ) as small, tc.tile_pool(
                name="ps_s", bufs=3, space="PSUM"
            ) as ps_s, tc.tile_pool(
                name="ps_o", bufs=1, space="PSUM"
            ) as ps_o:
                dist_t = []
                for kt in range(NKT):
                    d_tile = distp.tile([128, 1024], f32, tag=f"dist{kt}")
                    nc.sync.dma_start(out=d_tile[:], in_=dist_d[kt])
                    dist_t.append(d_tile)
                for h in range(NUM_HEADS):
                    sl = float(_SLOPES[h])
                    par = (h % 2) * 64
                    dt = h // 2
                    kts = _ktset(h)
                    n = len(kts)
                    o_ps = ps_o.tile([65, 1024], f32, tag="o2")
                    pts = [None] * n
                    for i in range(n + LAG):
                        if i < n:
                            kt = kts[i]
                            s2 = ps_s.tile([128, 1024], f32, tag="s2")
                            for half in range(2):
                                nc.tensor.matmul(
                                    s2[:, half * 512 : half * 512 + 512],
                                    lhsT=KT[par : par + 64, dt * T + kt * 128 : dt * T + kt * 128 + 128],
                                    rhs=QT[par : par + 64, dt * TQ + half * 512 : dt * TQ + half * 512 + 512],
                                    start=True,
                                    stop=True,
                                )
                            sscr = scr.tile([128, 1024], f32, tag="sscr")
                            nc.vector.tensor_add(sscr[:], s2[:], dist_t[kt][:])
                            pt = ptp.tile([128, 1024], bf16, tag="pt")
                            nc.scalar.activation(pt[:], sscr[:], Exp, bias=0.0, scale=sl)
                            pts[i] = (kt, pt)
                        j = i - LAG
                        if 0 <= j < n:
                            kt_j, pt_j = pts[j]
                            for half in range(2):
                                nc.tensor.matmul(
                                    o_ps[:, half * 512 : half * 512 + 512],
                                    lhsT=VP[:, kt_j * 1040 + h * 65 : kt_j * 1040 + h * 65 + 65],
                                    rhs=pt_j[:, half * 512 : half * 512 + 512],
                                    start=(j == 0),
                                    stop=(j == n - 1),
                                )
                            pts[j] = None
                    # normalize both q-chunks at once
                    # (reciprocal_approx_fast misreads PSUM sources - copy
                    # the denominator row to SBUF first)
                    den_sb = small.tile([1, 1024], f32, tag="den")
                    nc.scalar.copy(den_sb[:], o_ps[64:65, :])
                    rec = small.tile([1, 1024], f32, tag="rec")
                    nc.vector.reciprocal_approx_fast(rec[:], den_sb[:])
                    rec_bf = small.tile([1, 1024], bf16, tag="recbf")
                    nc.scalar.copy(rec_bf[:], rec[:])
                    rbc_ps = ps_s.tile([64, 1024], f32, tag="s2")
                    for half in range(2):
                        nc.tensor.matmul(
                            rbc_ps[:, half * 512 : half * 512 + 512],
                            lhsT=ones_sb[0:1, 0:64],
                            rhs=rec_bf[:, half * 512 : half * 512 + 512],
                            start=True,
                            stop=True,
                        )
                    rbc_sb = small.tile([64, 1024], bf16, tag="rbcsb")
                    nc.scalar.copy(rbc_sb[:], rbc_ps[:])
                    nc.vector.tensor_mul(
                        OT[par : par + 64, dt * TQ : dt * TQ + 1024],
                        o_ps[0:64, :],
                        rbc_sb[:],
                    )

        # ---- phase 3: output projection + bias ----
        with nc.named_scope("outproj"):
            with tc.tile_pool(name="ph3", bufs=1) as ph3, tc.tile_pool(
                name="ph3s", bufs=3
            ) as ph3s, tc.tile_pool(name="ps3", bufs=2, space="PSUM") as ps3:
                woT = ph3.tile([128, NDT * D_MODEL], bf16, tag="woT")
                for et in range(NDT):
                    nc.sync.dma_start(
                        out=woT[:, et * D_MODEL : (et + 1) * D_MODEL], in_=woT_d[et]
                    )
                for jt in range(NDT):
                    ps = ps3.tile([128, 1024], f32, tag="ps3")
                    for half in range(2):
                        for dm in range(NDT):
                            nc.tensor.matmul(
                                ps[:, half * 512 : half * 512 + 512],
                                lhsT=woT[:, dm * D_MODEL + jt * 128 : dm * D_MODEL + jt * 128 + 128],
                                rhs=OT[:, dm * TQ + half * 512 : dm * TQ + half * 512 + 512],
                                start=(dm == 0),
                                stop=(dm == NDT - 1),
                            )
                    osb = ph3s.tile([128, 1024], f32, tag="osb")
                    nc.scalar.activation(
                        osb[:], ps[:], Ident, bias=bo_sb[:, jt : jt + 1], scale=1.0
                    )
                    nc.sync.dma_start(out=out_d[jt], in_=osb[:])

    nc.finalize()
    _BUILT = nc
    return nc


def _pack(mat: np.ndarray, np_bf16) -> np.ndarray:
    """(n*128, C) f32 -> (n, 128, C) bf16."""
    n = mat.shape[0] // 128
    return np.ascontiguousarray(mat.reshape(n, 128, mat.shape[1]).astype(np_bf16))


def _install_trace_hook():
    """Recreate the NTFF profiling hook this container's image lacks.

    Mirrors trn_boot.py's _ntff_profile_via_ctypes against
    /opt/axon/libaxon_pjrt.so and registers it under the module path
    concourse.bass_utils expects. Also neutralizes upload_artifacts
    (no bucket access here)."""
    import contextlib
    import ctypes
    import sys
    import types

    if "antenv.axon_hooks" in sys.modules:
        return True
    so_path = "/opt/axon/libaxon_pjrt.so"
    try:
        lib = ctypes.CDLL(so_path)
    except OSError:
        return False
    if not hasattr(lib, "axon_start_nrt_profile"):
        return False
    lib.axon_start_nrt_profile.argtypes = [
        ctypes.POINTER(ctypes.c_int64),
        ctypes.c_size_t,
    ]
    lib.axon_start_nrt_profile.restype = ctypes.c_int64
    lib.axon_stop_nrt_profile.argtypes = [ctypes.c_char_p]
    lib.axon_stop_nrt_profile.restype = ctypes.c_int64

    @contextlib.contextmanager
    def _hook(output_dir, device_ids):
        import jax

        jax.devices()
        if device_ids:
            ids = (ctypes.c_int64 * len(device_ids))(*device_ids)
            rc = lib.axon_start_nrt_profile(ids, len(device_ids))
        else:
            rc = lib.axon_start_nrt_profile(None, 0)
        if rc != 0:
            raise RuntimeError(f"axon_start_nrt_profile rc={rc}")
        try:
            yield
        finally:
            n = lib.axon_stop_nrt_profile(str(output_dir).encode())
            print(f"profile: {n} file(s) written to {output_dir}")

    mod = types.ModuleType("antenv.axon_hooks")
    mod.get_axon_ntff_profile_hook = lambda: _hook
    mod.set_axon_ntff_profile_hook = lambda h: None
    sys.modules["antenv.axon_hooks"] = mod
    import antenv

    antenv.axon_hooks = mod

    import concourse.bass_utils as bu

    bu.upload_artifacts = lambda tmpdir: str(tmpdir)
    return True


def kernel(x, wq, wk, wv, wo, bo):
    import ml_dtypes

    from concourse.bass_utils import run_bass_kernel_spmd

    bf = ml_dtypes.bfloat16
    x = np.asarray(x, np.float32)
    wq = np.asarray(wq, np.float32)
    wk = np.asarray(wk, np.float32)
    wv = np.asarray(wv, np.float32)
    wo = np.asarray(wo, np.float32)
    bo = np.asarray(bo, np.float32)

    # fold 1/(8*slope_h) into wq rows of head h
    inv = np.repeat(1.0 / (8.0 * _SLOPES), HEAD_DIM).astype(np.float32)
    wq_s = wq * inv[:, None]

    wqT_p = _pack(wq_s.T, bf)
    wkT_p = _pack(wk.T, bf)
    wvT_p = _pack(wv.T, bf)
    woT_p = _pack(wo.T, bf)
    bo_t = np.ascontiguousarray(bo.reshape(NDT, 128).T.astype(np.float32))
    onesc = np.ones((128, 256), dtype=bf)

    nc = _build()

    in_maps = []
    for core in range(N_CORES):
        b, half = core // 2, core % 2
        q0 = half * TQ
        # roll the time axis so local queries are columns [0, TQ)
        xT_b = np.roll(x[b].T, -q0, axis=1)  # (1024, 2048)
        # dist2[kt][p, qc*512+j] = -|((kt*128+p+q0) mod T) - (q0 + qc*512 + j)|
        kl = (np.arange(T).reshape(NKT, 128, 1) + q0) % T
        ql = q0 + np.arange(TQ).reshape(1, 1, TQ)
        dist = -np.abs(kl - ql).astype(np.float32)  # (NKT, 128, 1024)
        in_maps.append(
            {
                "xT": _pack(xT_b, bf),
                "wqT": wqT_p,
                "wkT": wkT_p,
                "wvT": wvT_p,
                "woT": woT_p,
                "dist2": np.ascontiguousarray(dist),
                "bo_t": bo_t,
                "onesc": onesc,
            }
        )

    import os

    trace = os.environ.get("KERNEL_TRACE", "0") == "1"
    if trace and not _install_trace_hook():
        trace = False
    tmpdir = os.environ.get("KERNEL_TRACE_DIR") or None
    res = run_bass_kernel_spmd(
        nc, in_maps, core_ids=list(range(N_CORES)), trace=trace, tmpdir=tmpdir
    )
    if trace and res.exec_time_ns is not None:
        print(f"HW exec time: {res.exec_time_ns} ns")
        kernel.last_exec_time_ns = res.exec_time_ns
        kernel.last_results = res

    out = np.empty((B, T, D_MODEL), dtype=np.float32)
    for core in range(N_CORES):
        b, half = core // 2, core % 2
        o = np.asarray(res.results[core]["out"])  # (8, 128, 1024)
        out[b, half * TQ : (half + 1) * TQ, :] = o.reshape(D_MODEL, TQ).T
    return out
